# revision 1
# baseline (speedup 1.0000x reference)
"""DialogueGCN forward on 8 Trainium2 NeuronCores (Bass/Tile).

kernel(**inputs) -> np.ndarray [8192, 6] log-probs, matching reference().

Sharding: nodes row-sharded 1024/core. Edges sorted by destination; each core
owns the edges into its dst strip. Graph aggregation = dma_gather of per-edge
feature rows from DRAM + segment-sum as selection matmuls (128-edge blocks
against 32-dst groups). Cross-core: AllGather of h1 and h2. Dense attention is
row-sharded (queries = own strip, keys/values = full graph), computed in fp32
logits + fp16 softmax/PV, streaming keys in two halves.
"""
import numpy as np

import concourse.bass as bass
import concourse.tile as tile
import concourse.mybir as mybir
from concourse import bacc
from concourse.bass_utils import run_bass_kernel_spmd

f32 = mybir.dt.float32
f32r = mybir.dt.float32r
f16 = mybir.dt.float16
i16 = mybir.dt.int16

N, E, F, H, R, NB, NC = 8192, 680000, 200, 100, 8, 30, 6
CORES = 8
NPC = N // CORES            # 1024 dst rows per core
NGRP = NPC // 32            # 32-dst groups per core

AF = mybir.ActivationFunctionType
ALU = mybir.AluOpType
AX = mybir.AxisListType

_ker_cache = {}
_last_res = None


# ------------------------------------------------------------------ host prep
def _wrap_idx(idx):
    """int16 gather-index layout: j -> [j%16, j//16], replicated to 128 rows."""
    n = idx.shape[0]
    t = np.zeros((16, max(n // 16, 1)), np.int16)
    t[np.arange(n) % 16, np.arange(n) // 16] = idx.astype(np.int16)
    return np.tile(t, (8, 1))


def _prep(edge_index, edge_type):
    src = np.asarray(edge_index[0], np.int64)
    dst = np.asarray(edge_index[1], np.int64)
    et = np.asarray(edge_type, np.int64)

    deg = np.bincount(dst * R + et, minlength=N * R).astype(np.float64)
    inv = np.where(deg > 0, 1.0 / np.maximum(deg, 1.0), 0.0).astype(
        np.float32).reshape(N, R)

    core_of = dst // NPC
    grp_of = (dst % NPC) // 32

    def layout(nhalf):
        half = (et >= 4).astype(np.int64) if nhalf == 2 else np.zeros(E, np.int64)
        key = (core_of * NGRP + grp_of) * nhalf + half
        order = np.argsort(key, kind="stable")
        cnt = np.bincount(key, minlength=CORES * NGRP * nhalf)
        blocks = -(-cnt.reshape(CORES, NGRP * nhalf) // 128)
        B = blocks.max(axis=0)                      # static per (grp[,half])
        flat = np.zeros(CORES * NGRP * nhalf + 1, np.int64)
        flat[1:] = np.cumsum(cnt)
        return order, B, flat

    ord1, B1, flat1 = layout(2)
    ord2, B2, flat2 = layout(1)
    totB1, totB2 = int(B1.sum()), int(B2.sum())

    per_core = []
    for c in range(CORES):
        nA = int(B1[0::2].sum()) * 128
        nB = int(B1[1::2].sum()) * 128
        idxA = np.zeros(nA, np.int64)
        idxB = np.zeros(nB, np.int64)
        sel1 = np.zeros((totB1, 128, 32), np.float32)
        offA = offB = boff = 0
        for g in range(NGRP):
            for h in range(2):
                k = (c * NGRP + g) * 2 + h
                e = ord1[flat1[k]:flat1[k + 1]]
                n = e.shape[0]
                nb = int(B1[g * 2 + h])
                gi = src[e] * 4 + (et[e] - 4 * h)
                bi = np.arange(n)
                sel1[boff + bi // 128, bi % 128,
                     (dst[e] % NPC) - g * 32] = inv[dst[e], et[e]]
                if h == 0:
                    idxA[offA:offA + n] = gi
                    offA += nb * 128
                else:
                    idxB[offB:offB + n] = gi
                    offB += nb * 128
                boff += nb
        idx2 = np.zeros(totB2 * 128, np.int64)
        sel2 = np.zeros((totB2, 128, 32), np.float32)
        off = boff = 0
        for g in range(NGRP):
            k = c * NGRP + g
            e = ord2[flat2[k]:flat2[k + 1]]
            n = e.shape[0]
            nb = int(B2[g])
            bi = np.arange(n)
            sel2[boff + bi // 128, bi % 128, (dst[e] % NPC) - g * 32] = 1.0
            idx2[off:off + n] = src[e]
            off += nb * 128
            boff += nb
        per_core.append(dict(
            idxA=_wrap_idx(idxA), idxB=_wrap_idx(idxB), idx2=_wrap_idx(idx2),
            sel1=np.ascontiguousarray(sel1.transpose(1, 0, 2)).reshape(128, -1),
            sel2=np.ascontiguousarray(sel2.transpose(1, 0, 2)).reshape(128, -1)))

    meta = dict(B1=[(int(B1[g * 2]), int(B1[g * 2 + 1])) for g in range(NGRP)],
                B2=[int(b) for b in B2], totB1=totB1, totB2=totB2,
                lenA=per_core[0]["idxA"].shape[1] * 16,
                lenB=per_core[0]["idxB"].shape[1] * 16)
    return per_core, meta


# ------------------------------------------------------------------ program
def _build(meta, phase="full"):
    B1, B2 = meta["B1"], meta["B2"]
    totB1, totB2 = meta["totB1"], meta["totB2"]
    LA, LB = meta["lenA"], meta["lenB"]
    L2 = totB2 * 128
    MB1 = max(max(b) for b in B1)
    MB2 = max(B2)
    KH = N // 2                 # keys per attention half-pass

    nc = bacc.Bacc("TRN2", target_bir_lowering=False, debug=False,
                   num_devices=CORES)
    P = lambda n, s, d: nc.declare_dram_parameter(n, s, d, isOutput=False)

    xTd = P("xT", [F, N], f32)
    xd = P("x", [N, F], f32)
    basisd = P("basis", [NB, F * H], f32)
    compTd = P("compT", [NB, R], f32)
    rootwd = P("root_w", [F, H], f32)
    rootbd = P("root_b", [1, H], f32)
    gcreld = P("gc_rel_w", [H, H], f32)
    gcrelbd = P("gc_rel_b", [1, H], f32)
    gcrootd = P("gc_root_w", [H, H], f32)
    betawd = P("beta_w", [3 * H, 3 * H], f32)
    betabd = P("beta_b", [1, 3 * H], f32)
    linw16d = P("lin_w16", [3 * H, H], f16)
    linbd = P("lin_b", [1, H], f32)
    smaxw16d = P("smax_w16", [H, NC], f16)
    smaxbd = P("smax_b", [1, NC], f32)
    id32d = P("ident32", [128, 128], f32)
    id16d = P("ident16", [128, 128], f16)
    onesd = P("ones", [1, 512], f32)
    xTsd = P("xTs", [F, NPC], f32)
    idxAd = P("idxA", [128, LA // 16], i16)
    idxBd = P("idxB", [128, LB // 16], i16)
    idx2d = P("idx2", [128, L2 // 16], i16)
    sel1d = P("sel1", [128, totB1 * 32], f32)
    sel2d = P("sel2", [128, totB2 * 32], f32)

    outd = nc.declare_dram_parameter("out", [NPC, NC], f32, isOutput=True)
    dbgd = None
    if phase != "full":
        dbgd = nc.declare_dram_parameter("dbg", [N, 128], f32, isOutput=True)

    with tile.TileContext(nc, num_cores=CORES) as tc:
        with tc.tile_pool(name="dram", bufs=1, space="DRAM") as dram, \
             tc.tile_pool(name="persist", bufs=1) as pp:

            xwA = dram.tile([N * 4, 128], f32, tag="xwA")
            xwB = dram.tile([N * 4, 128], f32, tag="xwB")
            wtmp_d = dram.tile([R, F, H], f32, tag="wtmp")
            h1s_d = dram.tile([NPC, 128], f32, tag="h1s")
            h1f_d = dram.tile([N, 128], f32, tag="h1f")
            h2s_d = dram.tile([NPC, 128], f32, tag="h2s")
            h2f_d = dram.tile([N, 128], f32, tag="h2f")
            h2T_d = dram.tile([H, N], f32, tag="h2T_d")

            # --------- persistent small SBUF ---------
            xTs = pp.tile([100, 2, NPC], f32, tag="xTs")
            nc.sync.dma_start(xTs[:, 0, :], xTsd[0:100, :])
            nc.sync.dma_start(xTs[:, 1, :], xTsd[100:200, :])
            rootw = pp.tile([100, 2, H], f32, tag="rootw")
            nc.sync.dma_start(rootw[:, 0, :], rootwd[0:100, :])
            nc.sync.dma_start(rootw[:, 1, :], rootwd[100:200, :])
            rootb = pp.tile([1, H], f32, tag="rootb")
            nc.sync.dma_start(rootb[:], rootbd[:])
            gcrel = pp.tile([H, H], f32, tag="gcrel")
            nc.sync.dma_start(gcrel[:], gcreld[:])
            gcrelb = pp.tile([1, H], f32, tag="gcrelb")
            nc.sync.dma_start(gcrelb[:], gcrelbd[:])
            gcroot = pp.tile([H, H], f32, tag="gcroot")
            nc.sync.dma_start(gcroot[:], gcrootd[:])
            betaw = pp.tile([100, 3, 3, 100], f32, tag="betaw")
            nc.sync.dma_start(
                betaw[:], betawd[:].rearrange("(fc f) (gc g) -> f fc gc g",
                                              fc=3, gc=3))
            betab = pp.tile([1, 3 * H], f32, tag="betab")
            nc.sync.dma_start(betab[:], betabd[:])
            linw = pp.tile([100, 3, H], f16, tag="linw")
            nc.sync.dma_start(linw[:], linw16d[:].rearrange("(gc g) j -> g gc j",
                                                            gc=3))
            linb = pp.tile([1, H], f32, tag="linb")
            nc.sync.dma_start(linb[:], linbd[:])
            smaxw = pp.tile([H, NC], f16, tag="smaxw")
            nc.sync.dma_start(smaxw[:], smaxw16d[:])
            smaxb = pp.tile([1, NC], f32, tag="smaxb")
            nc.sync.dma_start(smaxb[:], smaxbd[:])
            id32 = pp.tile([128, 128], f32, tag="id32")
            nc.sync.dma_start(id32[:], id32d[:])
            id16 = pp.tile([128, 128], f16, tag="id16")
            nc.sync.dma_start(id16[:], id16d[:])
            ones = pp.tile([1, 512], f32, tag="ones")
            nc.sync.dma_start(ones[:], onesd[:])
            h1T = pp.tile([128, NPC], f32, tag="h1T")
            h2T = pp.tile([128, NPC], f32, tag="h2T")
            nc.vector.memset(h2T[:], 0.0)

            # ================= phase A: W then xW -> xwA/xwB =================
            with tc.tile_pool(name="pa", bufs=2) as pa, \
                 tc.tile_pool(name="pa1", bufs=1) as pa1, \
                 tc.tile_pool(name="psa", bufs=2, space="PSUM") as psa:
                basis_t = pa1.tile([NB, F * H], f32, tag="basis")
                nc.sync.dma_start(basis_t[:], basisd[:])
                compT = pa1.tile([NB, R], f32, tag="compT")
                nc.sync.dma_start(compT[:], compTd[:])
                wtmp_flat = wtmp_d[:].rearrange("r f h -> r (f h)")
                for t in range(F * H // 500):
                    pw = psa.tile([R, 500], f32, tag="pw")
                    nc.tensor.matmul(pw[:], compT[:],
                                     basis_t[:, t * 500:(t + 1) * 500],
                                     start=True, stop=True)
                    wb = pa.tile([R, 500], f32, tag="wb")
                    nc.vector.tensor_copy(wb[:], pw[:])
                    nc.sync.dma_start(wtmp_flat[:, t * 500:(t + 1) * 500], wb[:])
                wrhs = pa1.tile([100, 2, R * H], f32r, tag="wrhs")
                wld = pa1.tile([100, 2, R * H], f32, tag="wld")
                for c in range(2):
                    nc.sync.dma_start(
                        wld[:, c, :].rearrange("f (r h) -> f r h", h=H),
                        wtmp_d[:, c * 100:(c + 1) * 100, :].rearrange(
                            "r f h -> f r h"))
                nc.vector.tensor_copy(wrhs[:], wld[:])

                for nt in range(N // 128):
                    nsl = slice(nt * 128, (nt + 1) * 128)
                    xtl = pa.tile([100, 2, 128], f32, tag="xtl")
                    nc.sync.dma_start(xtl[:, 0, :], xTd[0:100, nsl])
                    nc.sync.dma_start(xtl[:, 1, :], xTd[100:200, nsl])
                    xtr = pa.tile([100, 2, 128], f32r, tag="xtr")
                    nc.vector.tensor_copy(xtr[:], xtl[:])
                    pxw = psa.tile([128, 2, 512], f32, tag="pxw")
                    for c in range(2):
                        for hf in range(2):
                            nc.tensor.matmul(pxw[:, hf, 0:400], xtr[:, c, :],
                                             wrhs[:, c,
                                                  hf * 400:(hf + 1) * 400],
                                             start=(c == 0), stop=(c == 1))
                    stage = pa.tile([128, R, 128], f32, tag="stage")
                    nc.vector.memset(stage[:, :, 100:128], 0.0)
                    for hf in range(2):
                        nc.vector.tensor_copy(
                            stage[:, hf * 4:(hf + 1) * 4, 0:100],
                            pxw[:, hf, 0:400].rearrange("p (r h) -> p r h",
                                                        h=H))
                    nc.sync.dma_start(
                        xwA[nt * 512:(nt + 1) * 512, :].rearrange(
                            "(p r) e -> p r e", r=4), stage[:, 0:4, :])
                    nc.sync.dma_start(
                        xwB[nt * 512:(nt + 1) * 512, :].rearrange(
                            "(p r) e -> p r e", r=4), stage[:, 4:8, :])

            if phase == "A":
                nc.sync.dma_start(
                    dbgd[:].rearrange("(b p) e -> p b e", p=128),
                    xwA[0:N, :].rearrange("(b p) e -> p b e", p=128))
            # ================= phases B/C: segment aggregation ===============
            def seg_stage(s):
                sel_d = sel1d if s == 1 else sel2d
                outT = h1T if s == 1 else h2T
                str_d = h1s_d if s == 1 else h2s_d
                full_d = h1f_d if s == 1 else h2f_d
                MB = MB1 if s == 1 else MB2
                with tc.tile_pool(name=f"pb{s}", bufs=3) as pb, \
                     tc.tile_pool(name=f"pq{s}", bufs=2) as pq, \
                     tc.tile_pool(name=f"pi{s}", bufs=1) as pi, \
                     tc.tile_pool(name=f"psb{s}", bufs=2, space="PSUM") as psb:
                    if s == 1:
                        idxA_t = pi.tile([128, LA // 16], i16, tag="idxA")
                        nc.sync.dma_start(idxA_t[:], idxAd[:])
                        idxB_t = pi.tile([128, LB // 16], i16, tag="idxB")
                        nc.sync.dma_start(idxB_t[:], idxBd[:])
                    else:
                        idx2_t = pi.tile([128, L2 // 16], i16, tag="idx2")
                        nc.sync.dma_start(idx2_t[:], idx2d[:])
                    GB = 8      # blocks per dma_gather (1024-idx HW limit)
                    boff = 0
                    offA = offB = off2 = 0
                    for ch in range(NPC // 128):
                        seq = []
                        for gg in range(4):
                            g = ch * 4 + gg
                            if s == 1:
                                seq += [(gg, 0, B1[g][0]), (gg, 1, B1[g][1])]
                            else:
                                seq.append((gg, 0, B2[g]))
                        seq = [t for t in seq if t[2] > 0]
                        totmm = sum(nb for _, _, nb in seq)
                        mmi = 0
                        ph = psb.tile([128, 128], f32, tag="ph")
                        for gg, h, nb in seq:
                            done = 0
                            while done < nb:
                                k = min(GB, nb - done)
                                mg = pb.tile([128, GB, 128], f32, tag="mg")
                                if s == 1 and h == 0:
                                    nc.gpsimd.dma_gather(
                                        mg[:, 0:k, :], xwA[:],
                                        idxA_t[:, offA // 16:
                                               (offA + k * 128) // 16],
                                        num_idxs=k * 128, num_idxs_reg=k * 128,
                                        elem_size=128)
                                    offA += k * 128
                                elif s == 1:
                                    nc.gpsimd.dma_gather(
                                        mg[:, 0:k, :], xwB[:],
                                        idxB_t[:, offB // 16:
                                               (offB + k * 128) // 16],
                                        num_idxs=k * 128, num_idxs_reg=k * 128,
                                        elem_size=128)
                                    offB += k * 128
                                else:
                                    nc.gpsimd.dma_gather(
                                        mg[:, 0:k, :], h1f_d[:],
                                        idx2_t[:, off2 // 16:
                                               (off2 + k * 128) // 16],
                                        num_idxs=k * 128, num_idxs_reg=k * 128,
                                        elem_size=128)
                                    off2 += k * 128
                                sel_t = pb.tile([128, GB, 32], f32, tag="sel")
                                nc.sync.dma_start(
                                    sel_t[:, 0:k, :],
                                    sel_d[:, boff * 32:(boff + k) * 32]
                                    .rearrange("p (b c) -> p b c", c=32))
                                for b in range(k):
                                    nc.tensor.matmul(
                                        ph[:, gg * 32:(gg + 1) * 32],
                                        mg[:, b, :], sel_t[:, b, :],
                                        start=(mmi == 0),
                                        stop=(s == 2 and mmi == totmm - 1))
                                    mmi += 1
                                boff += k
                                done += k
                        dsl = slice(ch * 128, (ch + 1) * 128)
                        if s == 1:
                            for c in range(2):
                                nc.tensor.matmul(ph[0:H, :], rootw[:, c, :],
                                                 xTs[:, c, dsl],
                                                 start=False, stop=False)
                            nc.tensor.matmul(ph[0:H, :], rootb[:],
                                             ones[:, 0:128], start=False,
                                             stop=True)
                            nc.vector.tensor_copy(outT[:, dsl], ph[:])
                        else:
                            a2 = pq.tile([128, 128], f32, tag="a2")
                            nc.vector.tensor_copy(a2[:], ph[:])
                            p2 = psb.tile([128, 128], f32, tag="p2")
                            nc.tensor.matmul(p2[0:H, :], gcrel[:], a2[0:H, :],
                                             start=True, stop=False)
                            nc.tensor.matmul(p2[0:H, :], gcroot[:],
                                             h1T[0:H, dsl], start=False,
                                             stop=False)
                            nc.tensor.matmul(p2[0:H, :], gcrelb[:],
                                             ones[:, 0:128], start=False,
                                             stop=True)
                            nc.vector.tensor_copy(outT[0:H, dsl], p2[0:H, :])
                        ptr = psb.tile([128, 128], f32, tag="ptr")
                        nc.tensor.matmul(ptr[:], outT[:, dsl], id32[:],
                                         is_transpose=True, start=True,
                                         stop=True)
                        nodem = pq.tile([128, 128], f32, tag="nodem")
                        nc.vector.tensor_copy(nodem[:], ptr[:])
                        nc.sync.dma_start(str_d[dsl, :], nodem[:])
                nc.gpsimd.collective_compute(
                    "AllGather", ALU.bypass,
                    replica_groups=[list(range(CORES))],
                    ins=[str_d[:].opt()], outs=[full_d[:].opt()])

            if phase != "A":
                seg_stage(1)
            if phase == "B":
                nc.sync.dma_start(
                    dbgd[:].rearrange("(b p) e -> p b e", p=128),
                    h1f_d[:].rearrange("(b p) e -> p b e", p=128))
            if phase in ("C", "full"):
                seg_stage(2)
            if phase == "C":
                nc.sync.dma_start(
                    dbgd[:].rearrange("(b p) e -> p b e", p=128),
                    h2f_d[:].rearrange("(b p) e -> p b e", p=128))

            # ================= phase D: attention =================
            if phase == "full":
                # h2f -> h2T_d (feature-major, DRAM)
                with tc.tile_pool(name="pt0", bufs=3) as pt0, \
                     tc.tile_pool(name="pst0", bufs=2, space="PSUM") as pst0:
                    for kb in range(N // 128):
                        blk = pt0.tile([128, 128], f32, tag="blk")
                        nc.sync.dma_start(blk[:], h2f_d[kb * 128:(kb + 1) * 128, :])
                        pt = pst0.tile([128, 128], f32, tag="pt")
                        nc.tensor.matmul(pt[:], blk[:], id32[:], is_transpose=True,
                                         start=True, stop=True)
                        h2tb = pt0.tile([100, 128], f32, tag="h2tb")
                        nc.vector.tensor_copy(h2tb[:], pt[0:100, :])
                        nc.sync.dma_start(h2T_d[:, kb * 128:(kb + 1) * 128], h2tb[:])

                # beforeT [100, 3(gc), NPC] from strip emoT
                befT = pp.tile([100, 3, NPC], f32, tag="befT")
                with tc.tile_pool(name="psf", bufs=2, space="PSUM") as psf:
                    emoTs = (xTs[:, 0, :], xTs[:, 1, :], h2T[0:H, :])
                    for gc in range(3):
                        for qh in range(NPC // 512):
                            qsl = slice(qh * 512, qh * 512 + 512)
                            pb_ = psf.tile([100, 512], f32, tag="pbef")
                            for fc in range(3):
                                nc.tensor.matmul(pb_[:], betaw[:, fc, gc, :],
                                                 emoTs[fc][:, qsl],
                                                 start=(fc == 0), stop=False)
                            nc.tensor.matmul(pb_[:],
                                             betab[:, gc * 100:(gc + 1) * 100],
                                             ones[:, 0:512], start=False, stop=True)
                            nc.vector.tensor_copy(befT[:, gc, qsl], pb_[:])

                # two half-passes over keys; flash combine
                NQT = NPC // 128
                m_st = pp.tile([128, 2, NQT], f32, tag="m_st")
                s_st = pp.tile([128, 2, NQT], f32, tag="s_st")
                em_st = pp.tile([128, 2, NQT, 3 * H], f32, tag="em_st")
                for kh in range(2):
                    with tc.tile_pool(name=f"pk{kh}", bufs=1) as pk, \
                         tc.tile_pool(name=f"pl{kh}", bufs=2) as pl, \
                         tc.tile_pool(name=f"psk{kh}", bufs=2, space="PSUM") as psk:
                        ksl_d = slice(kh * KH, (kh + 1) * KH)
                        keys = pk.tile([100, 3, KH], f32, tag="keys")
                        nc.sync.dma_start(keys[:, 0, :], xTd[0:100, ksl_d])
                        nc.sync.dma_start(keys[:, 1, :], xTd[100:200, ksl_d])
                        nc.sync.dma_start(keys[:, 2, :], h2T_d[:, ksl_d])
                        emoV = pk.tile([128, KH // 128, 3 * H], f16, tag="emoV")
                        nc.gpsimd.dma_start(
                            emoV[:, :, 0:F],
                            xd[ksl_d, :].rearrange("(kb p) f -> p kb f", p=128))
                        nc.gpsimd.dma_start(
                            emoV[:, :, F:F + H],
                            h2f_d[ksl_d, 0:H].rearrange("(kb p) f -> p kb f", p=128))
                        Srow = pk.tile([128, KH], f32, tag="Srow")
                        Prow = pk.tile([128, KH], f16, tag="Prow")
                        for qt in range(NQT):
                            qsl = slice(qt * 128, (qt + 1) * 128)
                            for kt in range(KH // 512):
                                ksl = slice(kt * 512, (kt + 1) * 512)
                                psS = psk.tile([128, 512], f32, tag="psS")
                                for fc in range(3):
                                    nc.tensor.matmul(psS[:], befT[:, fc, qsl],
                                                     keys[:, fc, ksl],
                                                     start=(fc == 0), stop=(fc == 2))
                                nc.vector.tensor_copy(Srow[:, ksl], psS[:])
                            mx = pl.tile([128, 1], f32, tag="mx")
                            nc.vector.reduce_max(mx[:], Srow[:], axis=AX.XYZW)
                            nc.vector.tensor_copy(m_st[:, kh, qt:qt + 1], mx[:])
                            nmx = pl.tile([128, 1], f32, tag="nmx")
                            nc.vector.tensor_scalar_mul(nmx[:], mx[:], -1.0)
                            ssum = pl.tile([128, 1], f32, tag="ssum")
                            nc.scalar.activation(Prow[:], Srow[:], AF.Exp,
                                                 bias=nmx[:], scale=1.0,
                                                 accum_out=ssum[:])
                            nc.vector.tensor_copy(s_st[:, kh, qt:qt + 1], ssum[:])
                            pem = psk.tile([128, 3 * H], f32, tag="pem")
                            nkb = KH // 128
                            for kb in range(nkb):
                                ptp = psk.tile([128, 128], f16, tag="ptp")
                                nc.tensor.matmul(ptp[:],
                                                 Prow[:, kb * 128:(kb + 1) * 128],
                                                 id16[:], is_transpose=True,
                                                 start=True, stop=True)
                                pts = pl.tile([128, 128], f16, tag="pts")
                                nc.vector.tensor_copy(pts[:], ptp[:])
                                nc.tensor.matmul(pem[:], pts[:], emoV[:, kb, :],
                                                 start=(kb == 0),
                                                 stop=(kb == nkb - 1))
                            nc.vector.tensor_copy(em_st[:, kh, qt, :], pem[:])

                # combine halves + head
                with tc.tile_pool(name="ph2", bufs=2) as ph2, \
                     tc.tile_pool(name="psh", bufs=2, space="PSUM") as psh:
                    for qt in range(NQT):
                        mm_ = ph2.tile([128, 1], f32, tag="mm_")
                        nc.vector.tensor_tensor(mm_[:], m_st[:, 0, qt:qt + 1],
                                                m_st[:, 1, qt:qt + 1], op=ALU.max)
                        al = ph2.tile([128, 2], f32, tag="al")
                        d0 = ph2.tile([128, 2], f32, tag="d0")
                        nc.vector.tensor_scalar(d0[:], m_st[:, :, qt], mm_[:], None,
                                                op0=ALU.subtract)
                        nc.scalar.activation(al[:], d0[:], AF.Exp)
                        sw = ph2.tile([128, 2], f32, tag="sw")
                        nc.vector.tensor_tensor(sw[:], s_st[:, :, qt], al[:],
                                                op=ALU.mult)
                        den = ph2.tile([128, 1], f32, tag="den")
                        nc.vector.reduce_sum(den[:], sw[:], axis=AX.XYZW)
                        rcp = ph2.tile([128, 1], f32, tag="rcp")
                        nc.vector.reciprocal(rcp[:], den[:])
                        e0 = ph2.tile([128, 3 * H], f32, tag="e0")
                        nc.vector.tensor_scalar(e0[:], em_st[:, 0, qt, :],
                                                al[:, 0:1], None, op0=ALU.mult)
                        e1 = ph2.tile([128, 3 * H], f32, tag="e1")
                        nc.vector.tensor_scalar(e1[:], em_st[:, 1, qt, :],
                                                al[:, 1:2], None, op0=ALU.mult)
                        es = ph2.tile([128, 3 * H], f32, tag="es")
                        nc.vector.tensor_tensor(es[:], e0[:], e1[:], op=ALU.add)
                        em2 = ph2.tile([128, 3 * H], f16, tag="em2")
                        nc.vector.tensor_scalar(em2[:], es[:], rcp[:], None,
                                                op0=ALU.mult)
                        # head: hiddenT = relu(lin_w.T @ em2.T + lin_b)
                        ph_ = psh.tile([100, 128], f32, tag="phid")
                        for gc in range(3):
                            pe2 = psh.tile([100, 128], f16, tag="pe2t")
                            nc.tensor.matmul(pe2[:],
                                             em2[:, gc * 100:(gc + 1) * 100],
                                             id16[:], is_transpose=True,
                                             start=True, stop=True)
                            e2t = ph2.tile([100, 128], f16, tag="e2t")
                            nc.vector.tensor_copy(e2t[:], pe2[:])
                            nc.tensor.matmul(ph_[:], linw[:, gc, :], e2t[:],
                                             start=(gc == 0), stop=False)
                        nc.tensor.matmul(ph_[:], linb[:], ones[:, 0:128],
                                         start=False, stop=True)
                        hidT = ph2.tile([100, 128], f16, tag="hidT")
                        nc.scalar.activation(hidT[:], ph_[:], AF.Relu)
                        plg = psh.tile([NC, 128], f32, tag="plg")
                        nc.tensor.matmul(plg[:], smaxw[:], hidT[:], start=True,
                                         stop=False)
                        nc.tensor.matmul(plg[:], smaxb[:], ones[:, 0:128],
                                         start=False, stop=True)
                        lgT = ph2.tile([NC, 128], f32, tag="lgT")
                        nc.vector.tensor_copy(lgT[:], plg[:])
                        plt = psh.tile([128, NC], f32, tag="plt")
                        nc.tensor.matmul(plt[:], lgT[:], id32[0:NC, 0:NC],
                                         is_transpose=True, start=True, stop=True)
                        lg = ph2.tile([128, NC], f32, tag="lg")
                        nc.vector.tensor_copy(lg[:], plt[:])
                        m6 = ph2.tile([128, 1], f32, tag="m6")
                        nc.vector.reduce_max(m6[:], lg[:], axis=AX.XYZW)
                        nm6 = ph2.tile([128, 1], f32, tag="nm6")
                        nc.vector.tensor_scalar_mul(nm6[:], m6[:], -1.0)
                        e6 = ph2.tile([128, NC], f32, tag="e6")
                        s6 = ph2.tile([128, 1], f32, tag="s6")
                        nc.scalar.activation(e6[:], lg[:], AF.Exp, bias=nm6[:],
                                             scale=1.0, accum_out=s6[:])
                        ls6 = ph2.tile([128, 1], f32, tag="ls6")
                        nc.scalar.activation(ls6[:], s6[:], AF.Ln)
                        sh = ph2.tile([128, 1], f32, tag="sh")
                        nc.vector.tensor_add(sh[:], m6[:], ls6[:])
                        outt = ph2.tile([128, NC], f32, tag="outt")
                        nc.vector.tensor_scalar(outt[:], lg[:], sh[:], None,
                                                op0=ALU.subtract)
                        nc.sync.dma_start(outd[qt * 128:(qt + 1) * 128, :], outt[:])

    nc.compile()
    return nc


# ------------------------------------------------------------------ entry
def kernel(x, edge_index, edge_norm, edge_type, basis, comp, root_w, root_b,
           gc_rel_w, gc_rel_b, gc_root_w, beta_w, beta_b, lin_w, lin_b,
           smax_w, smax_b):
    x = np.ascontiguousarray(np.asarray(x, np.float32))
    per_core, meta = _prep(edge_index, edge_type)

    import os
    phase = os.environ.get("KPHASE", "full")
    key = (phase, meta["totB1"], meta["totB2"], meta["lenA"], meta["lenB"],
           tuple(map(tuple, meta["B1"])), tuple(meta["B2"]))
    if key not in _ker_cache:
        _ker_cache[key] = _build(meta, phase)
    nc = _ker_cache[key]

    shared = dict(
        xT=np.ascontiguousarray(x.T),
        x=x,
        basis=np.ascontiguousarray(
            np.asarray(basis, np.float32).reshape(NB, F * H)),
        compT=np.ascontiguousarray(np.asarray(comp, np.float32).T),
        root_w=np.asarray(root_w, np.float32),
        root_b=np.asarray(root_b, np.float32).reshape(1, H),
        gc_rel_w=np.asarray(gc_rel_w, np.float32),
        gc_rel_b=np.asarray(gc_rel_b, np.float32).reshape(1, H),
        gc_root_w=np.asarray(gc_root_w, np.float32),
        beta_w=np.asarray(beta_w, np.float32),
        beta_b=np.asarray(beta_b, np.float32).reshape(1, 3 * H),
        lin_w16=np.asarray(lin_w, np.float16),
        lin_b=np.asarray(lin_b, np.float32).reshape(1, H),
        smax_w16=np.asarray(smax_w, np.float16),
        smax_b=np.asarray(smax_b, np.float32).reshape(1, NC),
        ident32=np.eye(128, dtype=np.float32),
        ident16=np.eye(128, dtype=np.float16),
        ones=np.ones((1, 512), np.float32),
    )
    in_maps = []
    for c in range(CORES):
        m = dict(shared)
        m["xTs"] = np.ascontiguousarray(x[c * NPC:(c + 1) * NPC, :].T)
        m.update(per_core[c])
        in_maps.append(m)

    res = run_bass_kernel_spmd(nc, in_maps, core_ids=list(range(CORES)),
                               trace_cores=[0])
    global _last_res
    _last_res = res
    if phase != "full":
        return [res.results[c]["dbg"] for c in range(CORES)]
    return np.concatenate([res.results[c]["out"] for c in range(CORES)], axis=0)



# revision 15
# speedup vs baseline: 1.0120x; 1.0120x over previous
"""DialogueGCN forward on 8 Trainium2 NeuronCores (Bass/Tile).

kernel(**inputs) -> np.ndarray [8192, 6] log-probs, matching reference().

Sharding: nodes row-sharded 1024/core. Edges sorted by destination; each core
owns the edges into its dst strip. Graph aggregation = dma_gather of per-edge
feature rows from DRAM + segment-sum as selection matmuls (128-edge blocks
against 32-dst groups). Cross-core: AllGather of h1 and h2. Dense attention is
row-sharded (queries = own strip, keys/values = full graph), computed in fp32
logits + fp16 softmax/PV, streaming keys in two halves.
"""
import numpy as np

import concourse.bass as bass
import concourse.tile as tile
import concourse.mybir as mybir
from concourse import bacc
from concourse.bass_utils import run_bass_kernel_spmd

f32 = mybir.dt.float32
f32r = mybir.dt.float32r
f16 = mybir.dt.float16
i16 = mybir.dt.int16

N, E, F, H, R, NB, NC = 8192, 680000, 200, 100, 8, 30, 6
CORES = 8
NPC = N // CORES            # 1024 dst rows per core
NGRP = NPC // 32            # 32-dst groups per core

AF = mybir.ActivationFunctionType
ALU = mybir.AluOpType
AX = mybir.AxisListType

_ker_cache = {}
_last_res = None


# ------------------------------------------------------------------ host prep
def _wrap_idx(idx):
    """int16 gather-index layout: j -> [j%16, j//16], replicated to 128 rows."""
    n = idx.shape[0]
    t = np.zeros((16, max(n // 16, 1)), np.int16)
    t[np.arange(n) % 16, np.arange(n) // 16] = idx.astype(np.int16)
    return np.tile(t, (8, 1))


def _prep(edge_index, edge_type):
    src = np.asarray(edge_index[0], np.int64)
    dst = np.asarray(edge_index[1], np.int64)
    et = np.asarray(edge_type, np.int64)

    deg = np.bincount(dst * R + et, minlength=N * R).astype(np.float64)
    inv = np.where(deg > 0, 1.0 / np.maximum(deg, 1.0), 0.0).astype(
        np.float32).reshape(N, R)

    core_of = dst // NPC
    grp_of = (dst % NPC) // 32

    def layout(nhalf):
        half = (et >= 4).astype(np.int64) if nhalf == 2 else np.zeros(E, np.int64)
        key = (core_of * NGRP + grp_of) * nhalf + half
        order = np.argsort(key, kind="stable")
        cnt = np.bincount(key, minlength=CORES * NGRP * nhalf)
        blocks = -(-cnt.reshape(CORES, NGRP * nhalf) // 128)
        B = blocks.max(axis=0)                      # static per (grp[,half])
        flat = np.zeros(CORES * NGRP * nhalf + 1, np.int64)
        flat[1:] = np.cumsum(cnt)
        return order, B, flat

    ord1, B1, flat1 = layout(2)
    ord2, B2, flat2 = layout(1)
    totB1, totB2 = int(B1.sum()), int(B2.sum())

    per_core = []
    for c in range(CORES):
        nA = int(B1[0::2].sum()) * 128
        nB = int(B1[1::2].sum()) * 128
        idxA = np.zeros(nA, np.int64)
        idxB = np.zeros(nB, np.int64)
        sel1 = np.zeros((totB1, 128, 32), np.float32)
        offA = offB = boff = 0
        for g in range(NGRP):
            for h in range(2):
                k = (c * NGRP + g) * 2 + h
                e = ord1[flat1[k]:flat1[k + 1]]
                n = e.shape[0]
                nb = int(B1[g * 2 + h])
                gi = src[e] * 4 + (et[e] - 4 * h)
                bi = np.arange(n)
                sel1[boff + bi // 128, bi % 128,
                     (dst[e] % NPC) - g * 32] = inv[dst[e], et[e]]
                if h == 0:
                    idxA[offA:offA + n] = gi
                    offA += nb * 128
                else:
                    idxB[offB:offB + n] = gi
                    offB += nb * 128
                boff += nb
        idx2 = np.zeros(totB2 * 128, np.int64)
        sel2 = np.zeros((totB2, 128, 32), np.float32)
        off = boff = 0
        for g in range(NGRP):
            k = c * NGRP + g
            e = ord2[flat2[k]:flat2[k + 1]]
            n = e.shape[0]
            nb = int(B2[g])
            bi = np.arange(n)
            sel2[boff + bi // 128, bi % 128, (dst[e] % NPC) - g * 32] = 1.0
            idx2[off:off + n] = src[e]
            off += nb * 128
            boff += nb
        per_core.append(dict(
            idxA=_wrap_idx(idxA), idxB=_wrap_idx(idxB), idx2=_wrap_idx(idx2),
            sel1=np.ascontiguousarray(sel1.transpose(1, 0, 2)).reshape(128, -1),
            sel2=np.ascontiguousarray(sel2.transpose(1, 0, 2)).reshape(128, -1)))

    meta = dict(B1=[(int(B1[g * 2]), int(B1[g * 2 + 1])) for g in range(NGRP)],
                B2=[int(b) for b in B2], totB1=totB1, totB2=totB2,
                lenA=per_core[0]["idxA"].shape[1] * 16,
                lenB=per_core[0]["idxB"].shape[1] * 16)
    return per_core, meta


# ------------------------------------------------------------------ program
def _build(meta, phase="full"):
    B1, B2 = meta["B1"], meta["B2"]
    totB1, totB2 = meta["totB1"], meta["totB2"]
    LA, LB = meta["lenA"], meta["lenB"]
    L2 = totB2 * 128
    MB1 = max(max(b) for b in B1)
    MB2 = max(B2)
    KH = N // 2                 # keys per attention half-pass

    nc = bacc.Bacc("TRN2", target_bir_lowering=False, debug=False,
                   num_devices=CORES)
    P = lambda n, s, d: nc.declare_dram_parameter(n, s, d, isOutput=False)

    xTd = P("xT", [F, N], f32)
    xd = P("x", [N, F], f32)
    basisd = P("basis", [NB, F * H], f32)
    compTd = P("compT", [NB, R], f32)
    rootwd = P("root_w", [F, H], f32)
    rootbd = P("root_b", [1, H], f32)
    gcreld = P("gc_rel_w", [H, H], f32)
    gcrelbd = P("gc_rel_b", [1, H], f32)
    gcrootd = P("gc_root_w", [H, H], f32)
    betawd = P("beta_w", [3 * H, 3 * H], f32)
    betabd = P("beta_b", [1, 3 * H], f32)
    linw16d = P("lin_w16", [3 * H, H], f16)
    linbd = P("lin_b", [1, H], f32)
    smaxw16d = P("smax_w16", [H, NC], f16)
    smaxbd = P("smax_b", [1, NC], f32)
    id32d = P("ident32", [128, 128], f32)
    id16d = P("ident16", [128, 128], f16)
    onesd = P("ones", [1, 512], f32)
    xTsd = P("xTs", [F, NPC], f32)
    idxAd = P("idxA", [128, LA // 16], i16)
    idxBd = P("idxB", [128, LB // 16], i16)
    idx2d = P("idx2", [128, L2 // 16], i16)
    sel1d = P("sel1", [128, totB1 * 32], f32)
    sel2d = P("sel2", [128, totB2 * 32], f32)

    outd = nc.declare_dram_parameter("out", [NPC, NC], f32, isOutput=True)
    dbgd = None
    if phase != "full":
        dbgd = nc.declare_dram_parameter("dbg", [N, 128], f32, isOutput=True)

    with tile.TileContext(nc, num_cores=CORES) as tc:
        with tc.tile_pool(name="dram", bufs=1, space="DRAM") as dram, \
             tc.tile_pool(name="persist", bufs=1) as pp:

            xwA = dram.tile([N * 4, 128], f32, tag="xwA")
            xwB = dram.tile([N * 4, 128], f32, tag="xwB")
            wtmp_d = dram.tile([R, F, H], f32, tag="wtmp")
            h1s_d = dram.tile([NPC, 128], f32, tag="h1s")
            h1f_d = dram.tile([N, 128], f32, tag="h1f")
            h2s_d = dram.tile([NPC, 128], f32, tag="h2s")
            h2f_d = dram.tile([N, 128], f32, tag="h2f")
            h2T1_d = dram.tile([H + 1, N], f32, tag="h2T1_d")
            cneg_d = dram.tile([1, NPC], f32, tag="cneg_d")

            # --------- persistent small SBUF ---------
            xTs = pp.tile([100, 2, NPC], f32, tag="xTs")
            nc.sync.dma_start(xTs[:, 0, :], xTsd[0:100, :])
            nc.sync.dma_start(xTs[:, 1, :], xTsd[100:200, :])
            rootw = pp.tile([100, 2, H], f32, tag="rootw")
            nc.sync.dma_start(rootw[:, 0, :], rootwd[0:100, :])
            nc.sync.dma_start(rootw[:, 1, :], rootwd[100:200, :])
            rootb = pp.tile([1, H], f32, tag="rootb")
            nc.sync.dma_start(rootb[:], rootbd[:])
            gcrel = pp.tile([H, H], f32, tag="gcrel")
            nc.sync.dma_start(gcrel[:], gcreld[:])
            gcrelb = pp.tile([1, H], f32, tag="gcrelb")
            nc.sync.dma_start(gcrelb[:], gcrelbd[:])
            gcroot = pp.tile([H, H], f32, tag="gcroot")
            nc.sync.dma_start(gcroot[:], gcrootd[:])
            betaw = pp.tile([100, 3, 3, 100], f32, tag="betaw")
            nc.sync.dma_start(
                betaw[:], betawd[:].rearrange("(fc f) (gc g) -> f fc gc g",
                                              fc=3, gc=3))
            betab = pp.tile([1, 3 * H], f32, tag="betab")
            nc.sync.dma_start(betab[:], betabd[:])
            linw = pp.tile([100, 3, H], f16, tag="linw")
            nc.sync.dma_start(linw[:], linw16d[:].rearrange("(gc g) j -> g gc j",
                                                            gc=3))
            linb = pp.tile([1, H], f32, tag="linb")
            nc.sync.dma_start(linb[:], linbd[:])
            smaxw = pp.tile([H, NC], f16, tag="smaxw")
            nc.sync.dma_start(smaxw[:], smaxw16d[:])
            smaxb = pp.tile([1, NC], f32, tag="smaxb")
            nc.sync.dma_start(smaxb[:], smaxbd[:])
            id32 = pp.tile([128, 128], f32, tag="id32")
            nc.sync.dma_start(id32[:], id32d[:])
            id16 = pp.tile([128, 128], f16, tag="id16")
            nc.sync.dma_start(id16[:], id16d[:])
            ones = pp.tile([1, 512], f32, tag="ones")
            nc.sync.dma_start(ones[:], onesd[:])
            h1T = pp.tile([128, NPC], f32, tag="h1T")
            h2T = pp.tile([128, NPC], f32, tag="h2T")
            nc.vector.memset(h2T[:], 0.0)

            # ================= phase A: W then xW -> xwA/xwB =================
            with tc.tile_pool(name="pa", bufs=2) as pa, \
                 tc.tile_pool(name="pa1", bufs=1) as pa1, \
                 tc.tile_pool(name="psa", bufs=2, space="PSUM") as psa:
                basis_t = pa1.tile([NB, F * H], f32, tag="basis")
                nc.sync.dma_start(basis_t[:], basisd[:])
                compT = pa1.tile([NB, R], f32, tag="compT")
                nc.sync.dma_start(compT[:], compTd[:])
                wtmp_flat = wtmp_d[:].rearrange("r f h -> r (f h)")
                for t in range(F * H // 500):
                    pw = psa.tile([R, 500], f32, tag="pw")
                    nc.tensor.matmul(pw[:], compT[:],
                                     basis_t[:, t * 500:(t + 1) * 500],
                                     start=True, stop=True)
                    wb = pa.tile([R, 500], f32, tag="wb")
                    nc.vector.tensor_copy(wb[:], pw[:])
                    nc.sync.dma_start(wtmp_flat[:, t * 500:(t + 1) * 500], wb[:])
                wrhs = pa1.tile([100, 2, R * H], f32r, tag="wrhs")
                wld = pa1.tile([100, 2, R * H], f32, tag="wld")
                for c in range(2):
                    nc.sync.dma_start(
                        wld[:, c, :].rearrange("f (r h) -> f r h", h=H),
                        wtmp_d[:, c * 100:(c + 1) * 100, :].rearrange(
                            "r f h -> f r h"))
                nc.vector.tensor_copy(wrhs[:], wld[:])

                for nt in range(N // 128):
                    nsl = slice(nt * 128, (nt + 1) * 128)
                    xtl = pa.tile([100, 2, 128], f32, tag="xtl")
                    nc.sync.dma_start(xtl[:, 0, :], xTd[0:100, nsl])
                    nc.sync.dma_start(xtl[:, 1, :], xTd[100:200, nsl])
                    xtr = pa.tile([100, 2, 128], f32r, tag="xtr")
                    nc.vector.tensor_copy(xtr[:], xtl[:])
                    pxw = psa.tile([128, 2, 512], f32, tag="pxw")
                    for c in range(2):
                        for hf in range(2):
                            nc.tensor.matmul(pxw[:, hf, 0:400], xtr[:, c, :],
                                             wrhs[:, c,
                                                  hf * 400:(hf + 1) * 400],
                                             start=(c == 0), stop=(c == 1))
                    stage = pa.tile([128, R, 128], f32, tag="stage")
                    nc.vector.memset(stage[:, :, 100:128], 0.0)
                    for hf in range(2):
                        nc.vector.tensor_copy(
                            stage[:, hf * 4:(hf + 1) * 4, 0:100],
                            pxw[:, hf, 0:400].rearrange("p (r h) -> p r h",
                                                        h=H))
                    nc.sync.dma_start(
                        xwA[nt * 512:(nt + 1) * 512, :].rearrange(
                            "(p r) e -> p r e", r=4), stage[:, 0:4, :])
                    nc.sync.dma_start(
                        xwB[nt * 512:(nt + 1) * 512, :].rearrange(
                            "(p r) e -> p r e", r=4), stage[:, 4:8, :])

            if phase == "A":
                nc.sync.dma_start(
                    dbgd[:].rearrange("(b p) e -> p b e", p=128),
                    xwA[0:N, :].rearrange("(b p) e -> p b e", p=128))
            # ================= phases B/C: segment aggregation ===============
            def seg_stage(s):
                sel_d = sel1d if s == 1 else sel2d
                outT = h1T if s == 1 else h2T
                str_d = h1s_d if s == 1 else h2s_d
                full_d = h1f_d if s == 1 else h2f_d
                MB = MB1 if s == 1 else MB2
                with tc.tile_pool(name=f"pb{s}", bufs=3) as pb, \
                     tc.tile_pool(name=f"pq{s}", bufs=2) as pq, \
                     tc.tile_pool(name=f"pi{s}", bufs=1) as pi, \
                     tc.tile_pool(name=f"psb{s}", bufs=2, space="PSUM") as psb:
                    if s == 1:
                        idxA_t = pi.tile([128, LA // 16], i16, tag="idxA")
                        nc.sync.dma_start(idxA_t[:], idxAd[:])
                        idxB_t = pi.tile([128, LB // 16], i16, tag="idxB")
                        nc.sync.dma_start(idxB_t[:], idxBd[:])
                    else:
                        idx2_t = pi.tile([128, L2 // 16], i16, tag="idx2")
                        nc.sync.dma_start(idx2_t[:], idx2d[:])
                    GB = 8      # blocks per dma_gather (1024-idx HW limit)
                    boff = 0
                    offA = offB = off2 = 0
                    for ch in range(NPC // 128):
                        seq = []
                        for gg in range(4):
                            g = ch * 4 + gg
                            if s == 1:
                                seq += [(gg, 0, B1[g][0]), (gg, 1, B1[g][1])]
                            else:
                                seq.append((gg, 0, B2[g]))
                        seq = [t for t in seq if t[2] > 0]
                        totmm = sum(nb for _, _, nb in seq)
                        mmi = 0
                        ph = psb.tile([128, 128], f32, tag="ph")
                        for gg, h, nb in seq:
                            done = 0
                            while done < nb:
                                k = min(GB, nb - done)
                                mg = pb.tile([128, GB, 128], f32, tag="mg")
                                if s == 1 and h == 0:
                                    nc.gpsimd.dma_gather(
                                        mg[:, 0:k, :], xwA[:],
                                        idxA_t[:, offA // 16:
                                               (offA + k * 128) // 16],
                                        num_idxs=k * 128, num_idxs_reg=k * 128,
                                        elem_size=128)
                                    offA += k * 128
                                elif s == 1:
                                    nc.gpsimd.dma_gather(
                                        mg[:, 0:k, :], xwB[:],
                                        idxB_t[:, offB // 16:
                                               (offB + k * 128) // 16],
                                        num_idxs=k * 128, num_idxs_reg=k * 128,
                                        elem_size=128)
                                    offB += k * 128
                                else:
                                    nc.gpsimd.dma_gather(
                                        mg[:, 0:k, :], h1f_d[:],
                                        idx2_t[:, off2 // 16:
                                               (off2 + k * 128) // 16],
                                        num_idxs=k * 128, num_idxs_reg=k * 128,
                                        elem_size=128)
                                    off2 += k * 128
                                sel_t = pb.tile([128, GB, 32], f32, tag="sel")
                                nc.sync.dma_start(
                                    sel_t[:, 0:k, :],
                                    sel_d[:, boff * 32:(boff + k) * 32]
                                    .rearrange("p (b c) -> p b c", c=32))
                                for b in range(k):
                                    nc.tensor.matmul(
                                        ph[:, gg * 32:(gg + 1) * 32],
                                        mg[:, b, :], sel_t[:, b, :],
                                        start=(mmi == 0),
                                        stop=(s == 2 and mmi == totmm - 1))
                                    mmi += 1
                                boff += k
                                done += k
                        dsl = slice(ch * 128, (ch + 1) * 128)
                        if s == 1:
                            for c in range(2):
                                nc.tensor.matmul(ph[0:H, :], rootw[:, c, :],
                                                 xTs[:, c, dsl],
                                                 start=False, stop=False)
                            nc.tensor.matmul(ph[0:H, :], rootb[:],
                                             ones[:, 0:128], start=False,
                                             stop=True)
                            nc.vector.tensor_copy(outT[:, dsl], ph[:])
                        else:
                            a2 = pq.tile([128, 128], f32, tag="a2")
                            nc.vector.tensor_copy(a2[:], ph[:])
                            p2 = psb.tile([128, 128], f32, tag="p2")
                            nc.tensor.matmul(p2[0:H, :], gcrel[:], a2[0:H, :],
                                             start=True, stop=False)
                            nc.tensor.matmul(p2[0:H, :], gcroot[:],
                                             h1T[0:H, dsl], start=False,
                                             stop=False)
                            nc.tensor.matmul(p2[0:H, :], gcrelb[:],
                                             ones[:, 0:128], start=False,
                                             stop=True)
                            nc.vector.tensor_copy(outT[0:H, dsl], p2[0:H, :])
                        ptr = psb.tile([128, 128], f32, tag="ptr")
                        nc.tensor.matmul(ptr[:], outT[:, dsl], id32[:],
                                         is_transpose=True, start=True,
                                         stop=True)
                        nodem = pq.tile([128, 128], f32, tag="nodem")
                        nc.vector.tensor_copy(nodem[:], ptr[:])
                        nc.sync.dma_start(str_d[dsl, :], nodem[:])
                nc.gpsimd.collective_compute(
                    "AllGather", ALU.bypass,
                    replica_groups=[list(range(CORES))],
                    ins=[str_d[:].opt()], outs=[full_d[:].opt()])

            if phase != "A":
                seg_stage(1)
            if phase == "B":
                nc.sync.dma_start(
                    dbgd[:].rearrange("(b p) e -> p b e", p=128),
                    h1f_d[:].rearrange("(b p) e -> p b e", p=128))
            if phase in ("C", "full"):
                seg_stage(2)
            if phase == "C":
                nc.sync.dma_start(
                    dbgd[:].rearrange("(b p) e -> p b e", p=128),
                    h2f_d[:].rearrange("(b p) e -> p b e", p=128))

            # ================= phase D: attention =================
            if phase == "full":
                NQT = NPC // 128
                # h2f -> h2T1_d ([100 feat + ones row], feature-major, DRAM)
                with tc.tile_pool(name="pt0", bufs=3) as pt0, \
                     tc.tile_pool(name="pst0", bufs=2, space="PSUM") as pst0:
                    for kb in range(N // 128):
                        blk = pt0.tile([128, 128], f32, tag="blk")
                        nc.sync.dma_start(blk[:], h2f_d[kb * 128:(kb + 1) * 128, :])
                        pt = pst0.tile([128, 128], f32, tag="pt")
                        nc.tensor.matmul(pt[:], blk[:], id32[:], is_transpose=True,
                                         start=True, stop=True)
                        h2tb = pt0.tile([100, 128], f32, tag="h2tb")
                        nc.vector.tensor_copy(h2tb[:], pt[0:100, :])
                        nc.sync.dma_start(h2T1_d[0:100, kb * 128:(kb + 1) * 128],
                                          h2tb[:])
                    for kc in range(N // 512):
                        nc.sync.dma_start(
                            h2T1_d[100:101, kc * 512:(kc + 1) * 512],
                            ones[:, 0:512])

                # emoV [node-major values, fp16]: cols 0..299 = [x, h2],
                # col 300 = 1 (denominator slot), 301..303 pad.
                emoV = pp.tile([128, N // 128, 304], f16, tag="emoV")
                nc.vector.memset(emoV[:, :, 301:304], 0.0)
                nc.vector.memset(emoV[:, :, 300:301], 1.0)
                nc.gpsimd.dma_start(
                    emoV[:, :, 0:F],
                    xd[:].rearrange("(kb p) f -> p kb f", p=128))
                nc.gpsimd.dma_start(
                    emoV[:, :, F:F + H],
                    h2f_d[:, 0:H].rearrange("(kb p) f -> p kb f", p=128))

                # beforeT [101, 3(gc), NPC] f32r; row 100 of gc2 = -(C+0.1)
                befT = pp.tile([101, 3, NPC], f32, tag="befT")
                befTr = pp.tile([100, 3, NPC], f32r, tag="befTr")
                with tc.tile_pool(name="psf", bufs=2, space="PSUM") as psf:
                    emoTs = (xTs[:, 0, :], xTs[:, 1, :], h2T[0:H, :])
                    for gc in range(3):
                        for qh in range(NPC // 512):
                            qsl = slice(qh * 512, qh * 512 + 512)
                            pb_ = psf.tile([100, 512], f32, tag="pbef")
                            for fc in range(3):
                                nc.tensor.matmul(pb_[:], betaw[:, fc, gc, :],
                                                 emoTs[fc][:, qsl],
                                                 start=(fc == 0), stop=False)
                            nc.tensor.matmul(pb_[:],
                                             betab[:, gc * 100:(gc + 1) * 100],
                                             ones[:, 0:512], start=False, stop=True)
                            nc.vector.tensor_copy(befT[0:100, gc, qsl], pb_[:])
                            nc.vector.tensor_copy(befTr[:, gc, qsl], pb_[:])

                # ---- max pass (q-major, f32r, streamed keys) ----
                with tc.tile_pool(name="pmx", bufs=3) as pmx, \
                     tc.tile_pool(name="pmx1", bufs=1) as pmx1, \
                     tc.tile_pool(name="psmx", bufs=4, space="PSUM") as psmx:
                    mxp = pmx1.tile([128, NQT, N // 512], f32, tag="mxp")
                    for kc in range(N // 512):
                        ksl = slice(kc * 512, (kc + 1) * 512)
                        ck = pmx.tile([101, 3, 512], f32r, tag="ck")
                        nc.sync.dma_start(ck[0:100, 0, :],
                                          xTd[0:100, ksl].bitcast(f32r))
                        nc.sync.dma_start(ck[0:100, 1, :],
                                          xTd[100:200, ksl].bitcast(f32r))
                        nc.sync.dma_start(ck[0:101, 2, :],
                                          h2T1_d[:, ksl].bitcast(f32r))
                        for qt in range(NQT):
                            qsl = slice(qt * 128, (qt + 1) * 128)
                            pS1 = psmx.tile([128, 512], f32, tag="pS1")
                            for fc in range(3):
                                nc.tensor.matmul(pS1[:], befTr[:, fc, qsl],
                                                 ck[0:100, fc, :],
                                                 start=(fc == 0), stop=(fc == 2))
                            nc.vector.reduce_max(mxp[:, qt, kc:kc + 1], pS1[:],
                                                 axis=AX.XYZW)
                    cng = pmx.tile([128, NQT], f32, tag="cng")
                    for qt in range(NQT):
                        mx1 = pmx.tile([128, 1], f32, tag="mx1")
                        nc.vector.reduce_max(mx1[:], mxp[:, qt, :], axis=AX.XYZW)
                        nc.vector.tensor_scalar(cng[:, qt:qt + 1], mx1[:],
                                                -1.0, -0.1,
                                                op0=ALU.mult, op1=ALU.add)
                    ptc = psmx.tile([NQT, 128], f32, tag="ptc")
                    nc.tensor.matmul(ptc[:], cng[:], id32[:], is_transpose=True,
                                     start=True, stop=True)
                    cngT = pmx.tile([NQT, 128], f32, tag="cngT")
                    nc.vector.tensor_copy(cngT[:], ptc[:])
                    nc.sync.dma_start(
                        cneg_d[0:1, :].rearrange("o (b p) -> b (o p)", b=NQT),
                        cngT[:])
                # round-trip through DRAM to move [128, NQT] -> [1, NPC];
                # DMA straight into partition row 100 of befT (DVE cannot
                # address partition offset 100).
                nc.sync.dma_start(befT[100:101, 2, :], cneg_d[0:1, :])

                # ---- main pass: S^T per key-tile, exp, PV accumulate ----
                em2all = pp.tile([128, NQT, 3 * H], f16, tag="em2all")
                with tc.tile_pool(name="pat", bufs=3) as pat, \
                     tc.tile_pool(name="pah", bufs=2) as pah, \
                     tc.tile_pool(name="psS", bufs=2, space="PSUM") as psSp, \
                     tc.tile_pool(name="psE", bufs=1, space="PSUM") as psEp:
                    for qh in range(NPC // 512):
                        qsl = slice(qh * 512, (qh + 1) * 512)
                        em2p = []
                        for qs in range(4):
                            em2p_t = psEp.tile([128, 304], f32,
                                               tag=f"em2p{qs}", name=f"em2p{qs}")
                            em2p.append(em2p_t)
                        for kc in range(N // 512):
                            ksl = slice(kc * 512, (kc + 1) * 512)
                            ck = pat.tile([101, 3, 512], f32, tag="ck2")
                            nc.sync.dma_start(ck[0:100, 0, :], xTd[0:100, ksl])
                            nc.sync.dma_start(ck[0:100, 1, :], xTd[100:200, ksl])
                            nc.sync.dma_start(ck[0:101, 2, :], h2T1_d[:, ksl])
                            for k4 in range(4):
                                kt = kc * 4 + k4
                                ktsl = slice(k4 * 128, (k4 + 1) * 128)
                                pS = psSp.tile([128, 512], f32, tag="pS")
                                for fc in range(3):
                                    kk = 101 if fc == 2 else 100
                                    nc.tensor.matmul(
                                        pS[:], ck[0:kk, fc, ktsl],
                                        befT[0:kk, fc, qsl],
                                        start=(fc == 0), stop=(fc == 2))
                                P16 = pat.tile([128, 512], f16, tag="P16")
                                nc.scalar.activation(P16[:], pS[:], AF.Exp)
                                for qs in range(4):
                                    nc.tensor.matmul(
                                        em2p[qs][:],
                                        P16[:, qs * 128:(qs + 1) * 128],
                                        emoV[:, kt, :],
                                        start=(kt == 0), stop=(kt == N // 128 - 1))
                        # ---- normalize into persistent em2all ----
                        for qs in range(4):
                            qt = qh * 4 + qs
                            rcp = pah.tile([128, 1], f32, tag="rcp")
                            nc.vector.reciprocal(rcp[:], em2p[qs][:, 300:301])
                            nc.vector.tensor_scalar(em2all[:, qt, :],
                                                    em2p[qs][:, 0:300],
                                                    rcp[:], None, op0=ALU.mult)
                # ---- head (per 128-query tile) ----
                with tc.tile_pool(name="ph2", bufs=2) as ph2, \
                     tc.tile_pool(name="psh", bufs=2, space="PSUM") as psh:
                    for qt in range(NQT):
                        em2 = em2all[:, qt, :]
                        ph_ = psh.tile([100, 128], f32, tag="phid")
                        for gc in range(3):
                            pe2 = psh.tile([100, 128], f16, tag="pe2t")
                            nc.tensor.matmul(pe2[:],
                                             em2[:, gc * 100:(gc + 1) * 100],
                                             id16[:], is_transpose=True,
                                             start=True, stop=True)
                            e2t = ph2.tile([100, 128], f16, tag="e2t")
                            nc.vector.tensor_copy(e2t[:], pe2[:])
                            nc.tensor.matmul(ph_[:], linw[:, gc, :], e2t[:],
                                             start=(gc == 0), stop=False)
                        nc.tensor.matmul(ph_[:], linb[:], ones[:, 0:128],
                                         start=False, stop=True)
                        hidT = ph2.tile([100, 128], f16, tag="hidT")
                        nc.scalar.activation(hidT[:], ph_[:], AF.Relu)
                        plg = psh.tile([NC, 128], f32, tag="plg")
                        nc.tensor.matmul(plg[:], smaxw[:], hidT[:], start=True,
                                         stop=False)
                        nc.tensor.matmul(plg[:], smaxb[:], ones[:, 0:128],
                                         start=False, stop=True)
                        lgT = ph2.tile([NC, 128], f32, tag="lgT")
                        nc.vector.tensor_copy(lgT[:], plg[:])
                        plt = psh.tile([128, NC], f32, tag="plt")
                        nc.tensor.matmul(plt[:], lgT[:], id32[0:NC, 0:NC],
                                         is_transpose=True, start=True, stop=True)
                        lg = ph2.tile([128, NC], f32, tag="lg")
                        nc.vector.tensor_copy(lg[:], plt[:])
                        m6 = ph2.tile([128, 1], f32, tag="m6")
                        nc.vector.reduce_max(m6[:], lg[:], axis=AX.XYZW)
                        nm6 = ph2.tile([128, 1], f32, tag="nm6")
                        nc.vector.tensor_scalar_mul(nm6[:], m6[:], -1.0)
                        e6 = ph2.tile([128, NC], f32, tag="e6")
                        s6 = ph2.tile([128, 1], f32, tag="s6")
                        nc.scalar.activation(e6[:], lg[:], AF.Exp, bias=nm6[:],
                                             scale=1.0, accum_out=s6[:])
                        ls6 = ph2.tile([128, 1], f32, tag="ls6")
                        nc.scalar.activation(ls6[:], s6[:], AF.Ln)
                        sh = ph2.tile([128, 1], f32, tag="sh")
                        nc.vector.tensor_add(sh[:], m6[:], ls6[:])
                        outt = ph2.tile([128, NC], f32, tag="outt")
                        nc.vector.tensor_scalar(outt[:], lg[:], sh[:], None,
                                                op0=ALU.subtract)
                        nc.sync.dma_start(outd[qt * 128:(qt + 1) * 128, :],
                                          outt[:])

    nc.compile()
    return nc


# ------------------------------------------------------------------ entry
def kernel(x, edge_index, edge_norm, edge_type, basis, comp, root_w, root_b,
           gc_rel_w, gc_rel_b, gc_root_w, beta_w, beta_b, lin_w, lin_b,
           smax_w, smax_b):
    x = np.ascontiguousarray(np.asarray(x, np.float32))
    per_core, meta = _prep(edge_index, edge_type)

    import os
    phase = os.environ.get("KPHASE", "full")
    key = (phase, meta["totB1"], meta["totB2"], meta["lenA"], meta["lenB"],
           tuple(map(tuple, meta["B1"])), tuple(meta["B2"]))
    if key not in _ker_cache:
        _ker_cache[key] = _build(meta, phase)
    nc = _ker_cache[key]

    shared = dict(
        xT=np.ascontiguousarray(x.T),
        x=x,
        basis=np.ascontiguousarray(
            np.asarray(basis, np.float32).reshape(NB, F * H)),
        compT=np.ascontiguousarray(np.asarray(comp, np.float32).T),
        root_w=np.asarray(root_w, np.float32),
        root_b=np.asarray(root_b, np.float32).reshape(1, H),
        gc_rel_w=np.asarray(gc_rel_w, np.float32),
        gc_rel_b=np.asarray(gc_rel_b, np.float32).reshape(1, H),
        gc_root_w=np.asarray(gc_root_w, np.float32),
        beta_w=np.asarray(beta_w, np.float32),
        beta_b=np.asarray(beta_b, np.float32).reshape(1, 3 * H),
        lin_w16=np.asarray(lin_w, np.float16),
        lin_b=np.asarray(lin_b, np.float32).reshape(1, H),
        smax_w16=np.asarray(smax_w, np.float16),
        smax_b=np.asarray(smax_b, np.float32).reshape(1, NC),
        ident32=np.eye(128, dtype=np.float32),
        ident16=np.eye(128, dtype=np.float16),
        ones=np.ones((1, 512), np.float32),
    )
    in_maps = []
    for c in range(CORES):
        m = dict(shared)
        m["xTs"] = np.ascontiguousarray(x[c * NPC:(c + 1) * NPC, :].T)
        m.update(per_core[c])
        in_maps.append(m)

    res = run_bass_kernel_spmd(nc, in_maps, core_ids=list(range(CORES)),
                               trace_cores=[0])
    global _last_res
    _last_res = res
    if phase != "full":
        return [res.results[c]["dbg"] for c in range(CORES)]
    return np.concatenate([res.results[c]["out"] for c in range(CORES)], axis=0)



# revision 21
# speedup vs baseline: 1.6506x; 1.6311x over previous
"""DialogueGCN forward on 8 Trainium2 NeuronCores (Bass/Tile).

kernel(**inputs) -> np.ndarray [8192, 6] log-probs, matching reference().

Sharding: nodes row-sharded 1024/core. Graph aggregation is fully dense:
per-relation weighted adjacency slices A_r [N, NPC] (fp16, host-built) are
streamed from DRAM and contracted against SBUF-resident per-relation node
features xW (fp16) on the PE array — no gathers. GraphConv stage streams an
exact-integer bf16 adjacency against a split-bf16 (hi+lo) h1g. Dense
attention is row-sharded (queries = own strip, keys = full graph) computed
transposed (keys on partitions): a f32r max pass finds row maxima, the
per-query offset is folded into the S matmul as an extra contraction row,
exp goes straight to fp16 P, and PV accumulates query-major with the softmax
denominator riding along as a free ones-column of the value matrix.
"""
import numpy as np
import ml_dtypes

import concourse.bass as bass
import concourse.tile as tile
import concourse.mybir as mybir
from concourse import bacc
from concourse.bass_utils import run_bass_kernel_spmd

f32 = mybir.dt.float32
f32r = mybir.dt.float32r
f16 = mybir.dt.float16
bf16 = mybir.dt.bfloat16
_bf16 = ml_dtypes.bfloat16

N, E, F, H, R, NB, NC = 8192, 680000, 200, 100, 8, 30, 6
CORES = 8
NPC = N // CORES            # 1024 dst rows per core
NSB = N // 128              # 64 source blocks

AF = mybir.ActivationFunctionType
ALU = mybir.AluOpType
AX = mybir.AxisListType

_ker_cache = {}
_last_res = None


# ------------------------------------------------------------------ host prep
def _prep(edge_index, edge_type):
    src = np.asarray(edge_index[0], np.int64)
    dst = np.asarray(edge_index[1], np.int64)
    et = np.asarray(edge_type, np.int64)

    deg = np.bincount(dst * R + et, minlength=N * R).astype(np.float64)
    inv = np.where(deg > 0, 1.0 / np.maximum(deg, 1.0), 0.0).astype(
        np.float32).reshape(N, R)

    per_core = []
    for c in range(CORES):
        m = (dst >= c * NPC) & (dst < (c + 1) * NPC)
        sl, dl, el = src[m], dst[m] - c * NPC, et[m]
        Ar = np.zeros((NSB, R, 128, NPC), np.float32)
        np.add.at(Ar, (sl // 128, el, sl % 128, dl), 1.0)
        Ar *= inv[c * NPC:(c + 1) * NPC, :].T[None, :, None, :]
        Au = np.zeros((N, NPC), np.float32)
        np.add.at(Au, (sl, dl), 1.0)
        per_core.append(dict(
            Arel=Ar.reshape(N * R, NPC).astype(np.float16),
            Aadj=Au.astype(_bf16)))
    return per_core


# ------------------------------------------------------------------ program
def _build():
    nc = bacc.Bacc("TRN2", target_bir_lowering=False, debug=False,
                   num_devices=CORES)
    P = lambda n, s, d: nc.declare_dram_parameter(n, s, d, isOutput=False)

    xTd = P("xT", [F, N], f32)
    xT2d = P("xT2", [100, 2, N], f32)
    x16d = P("x16", [N, F], f16)
    onesNd = P("onesN", [1, N], f32)
    basisd = P("basis", [NB, F * H], f32)
    compTd = P("compT", [NB, R], f32)
    rootwd = P("root_w", [F, H], f32)
    rootbd = P("root_b", [1, H], f32)
    gcreld = P("gc_rel_w", [H, H], f32)
    gcrelbd = P("gc_rel_b", [1, H], f32)
    gcrootd = P("gc_root_w", [H, H], f32)
    betawd = P("beta_w", [3 * H, 3 * H], f32)
    betabd = P("beta_b", [1, 3 * H], f32)
    linw16d = P("lin_w16", [3 * H, H], f16)
    linbd = P("lin_b", [1, H], f32)
    smaxw16d = P("smax_w16", [H, NC], f16)
    smaxbd = P("smax_b", [1, NC], f32)
    id32d = P("ident32", [128, 128], f32)
    id16d = P("ident16", [128, 128], f16)
    onesd = P("ones", [1, 512], f32)
    xTsd = P("xTs", [F, NPC], f32)
    Areld = P("Arel", [N * R, NPC], f16)
    Aadjd = P("Aadj", [N, NPC], bf16)

    outd = nc.declare_dram_parameter("out", [NPC, NC], f32, isOutput=True)

    with tile.TileContext(nc, num_cores=CORES) as tc:
        with tc.tile_pool(name="dram", bufs=1, space="DRAM") as dram, \
             tc.tile_pool(name="persist", bufs=1) as pp:

            wtmp_d = dram.tile([R, F, H], f32, tag="wtmp")
            h1gs_d = dram.tile([NPC, 200], f32, tag="h1gs")
            h1gf_d = dram.tile([N, 200], f32, tag="h1gf")
            h2s_d = dram.tile([NPC, 128], f32, tag="h2s")
            h2f_d = dram.tile([N, 128], f32, tag="h2f")
            h2T1_d = dram.tile([H + 1, N], f32, tag="h2T1_d")
            cneg_d = dram.tile([1, NPC], f32, tag="cneg_d")

            # --------- persistent small SBUF ---------
            xTs = pp.tile([100, 2, NPC], f32, tag="xTs")
            nc.sync.dma_start(xTs[:, 0, :], xTsd[0:100, :])
            nc.sync.dma_start(xTs[:, 1, :], xTsd[100:200, :])
            rootw = pp.tile([100, 2, H], f32, tag="rootw")
            nc.sync.dma_start(rootw[:, 0, :], rootwd[0:100, :])
            nc.sync.dma_start(rootw[:, 1, :], rootwd[100:200, :])
            rootb = pp.tile([1, H], f32, tag="rootb")
            nc.sync.dma_start(rootb[:], rootbd[:])
            gcrel = pp.tile([H, H], f32, tag="gcrel")
            nc.sync.dma_start(gcrel[:], gcreld[:])
            gcrelb = pp.tile([1, H], f32, tag="gcrelb")
            nc.sync.dma_start(gcrelb[:], gcrelbd[:])
            gcroot = pp.tile([H, H], f32, tag="gcroot")
            nc.sync.dma_start(gcroot[:], gcrootd[:])
            betaw = pp.tile([100, 3, 3, 100], f32, tag="betaw")
            nc.sync.dma_start(
                betaw[:], betawd[:].rearrange("(fc f) (gc g) -> f fc gc g",
                                              fc=3, gc=3))
            betab = pp.tile([1, 3 * H], f32, tag="betab")
            nc.sync.dma_start(betab[:], betabd[:])
            linw = pp.tile([100, 3, H], f16, tag="linw")
            nc.sync.dma_start(linw[:], linw16d[:].rearrange("(gc g) j -> g gc j",
                                                            gc=3))
            linb = pp.tile([1, H], f32, tag="linb")
            nc.sync.dma_start(linb[:], linbd[:])
            smaxw = pp.tile([H, NC], f16, tag="smaxw")
            nc.sync.dma_start(smaxw[:], smaxw16d[:])
            smaxb = pp.tile([1, NC], f32, tag="smaxb")
            nc.sync.dma_start(smaxb[:], smaxbd[:])
            id32 = pp.tile([128, 128], f32, tag="id32")
            nc.sync.dma_start(id32[:], id32d[:])
            id16 = pp.tile([128, 128], f16, tag="id16")
            nc.sync.dma_start(id16[:], id16d[:])
            ones = pp.tile([1, 512], f32, tag="ones")
            nc.sync.dma_start(ones[:], onesd[:])
            h1T = pp.tile([128, NPC], f32, tag="h1T")
            h2T = pp.tile([128, NPC], f32, tag="h2T")
            nc.vector.memset(h1T[:], 0.0)
            nc.vector.memset(h2T[:], 0.0)
            wrhs = pp.tile([100, 2, R * H], f32r, tag="wrhs")

            # ===== W = comp @ basis (per-relation projection matrices) =====
            with tc.tile_pool(name="pwb", bufs=2) as pwb, \
                 tc.tile_pool(name="pwb1", bufs=1) as pwb1, \
                 tc.tile_pool(name="pswb", bufs=2, space="PSUM") as pswb:
                basis_t = pwb1.tile([NB, F * H], f32, tag="basis")
                nc.sync.dma_start(basis_t[:], basisd[:])
                compT = pwb1.tile([NB, R], f32, tag="compT")
                nc.sync.dma_start(compT[:], compTd[:])
                wtmp_flat = wtmp_d[:].rearrange("r f h -> r (f h)")
                for t in range(F * H // 500):
                    pw = pswb.tile([R, 500], f32, tag="pw")
                    nc.tensor.matmul(pw[:], compT[:],
                                     basis_t[:, t * 500:(t + 1) * 500],
                                     start=True, stop=True)
                    wb = pwb.tile([R, 500], f32, tag="wb")
                    nc.vector.tensor_copy(wb[:], pw[:])
                    nc.sync.dma_start(wtmp_flat[:, t * 500:(t + 1) * 500], wb[:])
                wld = pwb1.tile([100, 2, R * H], f32, tag="wld")
                for c in range(2):
                    nc.sync.dma_start(
                        wld[:, c, :].rearrange("f (r h) -> f r h", h=H),
                        wtmp_d[:, c * 100:(c + 1) * 100, :].rearrange(
                            "r f h -> f r h"))
                nc.vector.tensor_copy(wrhs[:], wld[:])

            # ===== fused: xW into SBUF (fp16) + dense stage-1 =====
            # h1 = sum_r A_r^T (x @ W_r) + x @ root_w + root_b, with A_r
            # streamed fp16 [128 src x NPC dst] tiles (weights = inv deg).
            with tc.tile_pool(name="pa", bufs=3) as pa, \
                 tc.tile_pool(name="pxw", bufs=1) as pxwp, \
                 tc.tile_pool(name="ps1", bufs=6) as ps1, \
                 tc.tile_pool(name="psa", bufs=2, space="PSUM") as psa, \
                 tc.tile_pool(name="psh1", bufs=1, space="PSUM") as psh1:
                xw16 = pxwp.tile([128, NSB, R, 100], f16, tag="xw16")
                h1p = []
                for dh in range(2):
                    h1p_t = psh1.tile([100, 512], f32,
                                      tag=f"h1p{dh}", name=f"h1p{dh}")
                    h1p.append(h1p_t)
                for sb in range(NSB):
                    nsl = slice(sb * 128, (sb + 1) * 128)
                    xtl = pa.tile([100, 2, 128], f32, tag="xtl")
                    nc.sync.dma_start(xtl[:], xT2d[:, :, nsl])
                    xtr = pa.tile([100, 2, 128], f32r, tag="xtr")
                    nc.vector.tensor_copy(xtr[:], xtl[:])
                    pxw = psa.tile([128, 2, 512], f32, tag="pxw")
                    for c in range(2):
                        for hf in range(2):
                            nc.tensor.matmul(pxw[:, hf, 0:400], xtr[:, c, :],
                                             wrhs[:, c,
                                                  hf * 400:(hf + 1) * 400],
                                             start=(c == 0), stop=(c == 1))
                    for hf in range(2):
                        nc.vector.tensor_copy(
                            xw16[:, sb, hf * 4:(hf + 1) * 4, :],
                            pxw[:, hf, 0:400].rearrange("p (r h) -> p r h",
                                                        h=H))
                    for r in range(R):
                        At = ps1.tile([128, NPC], f16, tag="Ar")
                        row = (sb * R + r) * 128
                        nc.sync.dma_start(At[:], Areld[row:row + 128, :])
                        for dh in range(2):
                            dsl = slice(dh * 512, (dh + 1) * 512)
                            nc.tensor.matmul(h1p[dh][:], xw16[:, sb, r, :],
                                             At[:, dsl],
                                             start=(sb == 0 and r == 0),
                                             stop=False)
                for dh in range(2):
                    dsl = slice(dh * 512, (dh + 1) * 512)
                    for c in range(2):
                        nc.tensor.matmul(h1p[dh][:], rootw[:, c, :],
                                         xTs[:, c, dsl],
                                         start=False, stop=False)
                    nc.tensor.matmul(h1p[dh][:], rootb[:], ones[:, 0:512],
                                     start=False, stop=True)
                    nc.vector.tensor_copy(h1T[0:H, dsl], h1p[dh][:])

            # ===== stage 2: h2 = A^T (h1 gc_rel_w) + gc_rel_b + h1 gc_root_w
            # A exact-int bf16; h1g split to bf16 hi+lo before the AllGather.
            with tc.tile_pool(name="s2", bufs=3) as s2p, \
                 tc.tile_pool(name="s2a", bufs=1) as s2a, \
                 tc.tile_pool(name="s2q", bufs=2) as s2q, \
                 tc.tile_pool(name="pss2", bufs=2, space="PSUM") as pss2, \
                 tc.tile_pool(name="psag", bufs=1, space="PSUM") as psag:
                h1gT = s2a.tile([128, NPC], f32, tag="h1gT")
                nc.vector.memset(h1gT[:], 0.0)
                for dh in range(NPC // 512):
                    dsl = slice(dh * 512, (dh + 1) * 512)
                    pg = pss2.tile([100, 512], f32, tag="pg")
                    nc.tensor.matmul(pg[:], gcrel[:], h1T[0:H, dsl],
                                     start=True, stop=True)
                    nc.vector.tensor_copy(h1gT[0:100, dsl], pg[:])
                # node-major hi/lo split of own strip, one AllGather
                for t in range(NPC // 128):
                    tsl = slice(t * 128, (t + 1) * 128)
                    ptg = pss2.tile([128, 128], f32, tag="ptg")
                    nc.tensor.matmul(ptg[:], h1gT[:, tsl], id32[:],
                                     is_transpose=True, start=True,
                                     stop=True)
                    nhl = s2q.tile([128, 200], f32, tag="nhl")
                    hi16 = s2q.tile([128, 100], bf16, tag="hi16")
                    nc.vector.tensor_copy(hi16[:], ptg[:, 0:100])
                    nc.vector.tensor_copy(nhl[:, 0:100], hi16[:])
                    nc.vector.tensor_tensor(nhl[:, 100:200], ptg[:, 0:100],
                                            nhl[:, 0:100], op=ALU.subtract)
                    nc.vector.tensor_copy(nhl[:, 0:100], ptg[:, 0:100])
                    nc.sync.dma_start(h1gs_d[tsl, :], nhl[:])
                nc.gpsimd.collective_compute(
                    "AllGather", ALU.bypass,
                    replica_groups=[list(range(CORES))],
                    ins=[h1gs_d[:].opt()], outs=[h1gf_d[:].opt()])
                h1ghl = s2a.tile([128, NSB, 200], bf16, tag="h1ghl")
                with tc.tile_pool(name="s2t", bufs=1) as s2t:
                    h1g_l = s2t.tile([128, NSB, 200], f32, tag="h1g_l")
                    nc.sync.dma_start(
                        h1g_l[:],
                        h1gf_d[:].rearrange("(b p) f -> p b f", p=128))
                    nc.vector.tensor_copy(h1ghl[:], h1g_l[:])
                aggT = []
                for dh in range(2):
                    aggT_t = psag.tile([100, 512], f32,
                                       tag=f"aggT{dh}", name=f"aggT{dh}")
                    aggT.append(aggT_t)
                for sb in range(NSB):
                    At = s2p.tile([128, NPC], bf16, tag="At")
                    nc.sync.dma_start(
                        At[:], Aadjd[sb * 128:(sb + 1) * 128, :])
                    for dh in range(2):
                        dsl = slice(dh * 512, (dh + 1) * 512)
                        nc.tensor.matmul(aggT[dh][:], h1ghl[:, sb, 0:100],
                                         At[:, dsl],
                                         start=(sb == 0), stop=False)
                        nc.tensor.matmul(aggT[dh][:], h1ghl[:, sb, 100:200],
                                         At[:, dsl],
                                         start=False, stop=False)
                for dh in range(2):
                    dsl = slice(dh * 512, (dh + 1) * 512)
                    nc.tensor.matmul(aggT[dh][:], gcroot[:],
                                     h1T[0:H, dsl],
                                     start=False, stop=False)
                    nc.tensor.matmul(aggT[dh][:], gcrelb[:],
                                     ones[:, 0:512],
                                     start=False, stop=True)
                    nc.vector.tensor_copy(h2T[0:H, dsl], aggT[dh][:])
                # h2 node-major + AllGather
                for ch in range(NPC // 128):
                    dsl = slice(ch * 128, (ch + 1) * 128)
                    ptr = pss2.tile([128, 128], f32, tag="ptr2")
                    nc.tensor.matmul(ptr[:], h2T[:, dsl], id32[:],
                                     is_transpose=True, start=True,
                                     stop=True)
                    nodem = s2q.tile([128, 128], f32, tag="nodem2")
                    nc.vector.tensor_copy(nodem[:], ptr[:])
                    nc.sync.dma_start(h2s_d[dsl, :], nodem[:])
            nc.gpsimd.collective_compute(
                "AllGather", ALU.bypass,
                replica_groups=[list(range(CORES))],
                ins=[h2s_d[:].opt()], outs=[h2f_d[:].opt()])

            # ================= attention =================
            NQT = NPC // 128
            with tc.tile_pool(name="pd", bufs=1) as pd:
                # h2f -> h2T1_d ([100 feat + ones row], feature-major, DRAM)
                with tc.tile_pool(name="pt0", bufs=3) as pt0, \
                     tc.tile_pool(name="pst0", bufs=2, space="PSUM") as pst0:
                    nc.sync.dma_start(h2T1_d[100:101, :], onesNd[:])
                    for kc in range(N // 512):
                        h2tb = pt0.tile([100, 4, 128], f32, tag="h2tb")
                        for k4 in range(4):
                            kb = kc * 4 + k4
                            blk = pt0.tile([128, 128], f32, tag="blk")
                            nc.sync.dma_start(
                                blk[:], h2f_d[kb * 128:(kb + 1) * 128, :])
                            pt = pst0.tile([128, 128], f32, tag="pt")
                            nc.tensor.matmul(pt[:], blk[:], id32[:],
                                             is_transpose=True,
                                             start=True, stop=True)
                            nc.vector.tensor_copy(h2tb[:, k4, :], pt[0:100, :])
                        nc.scalar.dma_start(
                            h2T1_d[0:100, kc * 512:(kc + 1) * 512],
                            h2tb[:].rearrange("f k p -> f (k p)"))

                # emoV [node-major values, fp16]: cols 0..299 = [x, h2],
                # col 300 = 1 (denominator slot), 301..303 pad.
                emoV = pd.tile([128, N // 128, 304], f16, tag="emoV")
                nc.vector.memset(emoV[:, :, 301:304], 0.0)
                nc.vector.memset(emoV[:, :, 300:301], 1.0)
                nc.sync.dma_start(
                    emoV[:, :, 0:F],
                    x16d[:].rearrange("(kb p) f -> p kb f", p=128))
                with tc.tile_pool(name="ph2l", bufs=1) as ph2l:
                    h2l = ph2l.tile([128, N // 128, H], f32, tag="h2l")
                    nc.sync.dma_start(
                        h2l[:],
                        h2f_d[:, 0:H].rearrange("(kb p) f -> p kb f", p=128))
                    nc.vector.tensor_copy(emoV[:, :, F:F + H], h2l[:])

                # beforeT [101, 3(gc), NPC]; row 100 of gc2 = -(C+0.1)
                befT = pd.tile([101, 3, NPC], f32, tag="befT")
                befTr = pd.tile([100, 3, NPC], f32r, tag="befTr")
                with tc.tile_pool(name="psf", bufs=2, space="PSUM") as psf:
                    emoTs = (xTs[:, 0, :], xTs[:, 1, :], h2T[0:H, :])
                    for gc in range(3):
                        for qh in range(NPC // 512):
                            qsl = slice(qh * 512, qh * 512 + 512)
                            pb_ = psf.tile([100, 512], f32, tag="pbef")
                            for fc in range(3):
                                nc.tensor.matmul(pb_[:], betaw[:, fc, gc, :],
                                                 emoTs[fc][:, qsl],
                                                 start=(fc == 0), stop=False)
                            nc.tensor.matmul(pb_[:],
                                             betab[:, gc * 100:(gc + 1) * 100],
                                             ones[:, 0:512], start=False,
                                             stop=True)
                            nc.vector.tensor_copy(befT[0:100, gc, qsl], pb_[:])
                            nc.vector.tensor_copy(befTr[:, gc, qsl], pb_[:])

                # ---- max pass (q-major, f32r, streamed keys) ----
                with tc.tile_pool(name="pmx", bufs=3) as pmx, \
                     tc.tile_pool(name="pmx1", bufs=1) as pmx1, \
                     tc.tile_pool(name="psmx", bufs=7, space="PSUM") as psmx, \
                     tc.tile_pool(name="psmc", bufs=1, space="PSUM") as psmc:
                    mxp = pmx1.tile([128, NQT, N // 512], f32, tag="mxp")
                    for kc in range(N // 512):
                        ksl = slice(kc * 512, (kc + 1) * 512)
                        ck = pmx.tile([101, 3, 512], f32r, tag="ck")
                        nc.sync.dma_start(ck[0:100, 0, :],
                                          xTd[0:100, ksl].bitcast(f32r))
                        nc.sync.dma_start(ck[0:100, 1, :],
                                          xTd[100:200, ksl].bitcast(f32r))
                        nc.sync.dma_start(ck[0:101, 2, :],
                                          h2T1_d[:, ksl].bitcast(f32r))
                        for qt in range(NQT):
                            qsl = slice(qt * 128, (qt + 1) * 128)
                            pS1 = psmx.tile([128, 512], f32, tag="pS1")
                            for fc in range(3):
                                nc.tensor.matmul(pS1[:], befTr[:, fc, qsl],
                                                 ck[0:100, fc, :],
                                                 start=(fc == 0),
                                                 stop=(fc == 2))
                            nc.vector.reduce_max(mxp[:, qt, kc:kc + 1], pS1[:],
                                                 axis=AX.XYZW)
                    cng = pmx.tile([128, NQT], f32, tag="cng")
                    for qt in range(NQT):
                        mx1 = pmx.tile([128, 1], f32, tag="mx1")
                        nc.vector.reduce_max(mx1[:], mxp[:, qt, :],
                                             axis=AX.XYZW)
                        nc.vector.tensor_scalar(cng[:, qt:qt + 1], mx1[:],
                                                -1.0, -0.1,
                                                op0=ALU.mult, op1=ALU.add)
                    ptc = psmc.tile([NQT, 128], f32, tag="ptc")
                    nc.tensor.matmul(ptc[:], cng[:], id32[:],
                                     is_transpose=True, start=True, stop=True)
                    cngT = pmx.tile([NQT, 128], f32, tag="cngT")
                    nc.vector.tensor_copy(cngT[:], ptc[:])
                    nc.sync.dma_start(
                        cneg_d[0:1, :].rearrange("o (b p) -> b (o p)", b=NQT),
                        cngT[:])
                # DRAM round trip moves [128, NQT] -> [1, NPC]; DMA straight
                # into partition row 100 of befT (DVE cannot address it).
                nc.sync.dma_start(befT[100:101, 2, :], cneg_d[0:1, :])

                # ---- main pass: S^T per key-tile, exp, PV accumulate ----
                em2all = pd.tile([128, NQT, 3 * H], f16, tag="em2all")
                with tc.tile_pool(name="pat", bufs=3) as pat, \
                     tc.tile_pool(name="pah", bufs=2) as pah, \
                     tc.tile_pool(name="psS", bufs=2, space="PSUM") as psSp, \
                     tc.tile_pool(name="psE", bufs=1, space="PSUM") as psEp:
                    for qh in range(NPC // 512):
                        qsl = slice(qh * 512, (qh + 1) * 512)
                        em2p = []
                        for qs in range(4):
                            em2p_t = psEp.tile([128, 304], f32,
                                               tag=f"em2p{qs}",
                                               name=f"em2p{qs}")
                            em2p.append(em2p_t)
                        for kc in range(N // 512):
                            ksl = slice(kc * 512, (kc + 1) * 512)
                            ck = pat.tile([101, 3, 512], f32, tag="ck2")
                            nc.sync.dma_start(ck[0:100, 0, :], xTd[0:100, ksl])
                            nc.sync.dma_start(ck[0:100, 1, :],
                                              xTd[100:200, ksl])
                            nc.sync.dma_start(ck[0:101, 2, :], h2T1_d[:, ksl])
                            for k4 in range(4):
                                kt = kc * 4 + k4
                                ktsl = slice(k4 * 128, (k4 + 1) * 128)
                                pS = psSp.tile([128, 512], f32, tag="pS")
                                for fc in range(3):
                                    kk = 101 if fc == 2 else 100
                                    nc.tensor.matmul(
                                        pS[:], ck[0:kk, fc, ktsl],
                                        befT[0:kk, fc, qsl],
                                        start=(fc == 0), stop=(fc == 2))
                                P16 = pat.tile([128, 512], f16, tag="P16")
                                nc.scalar.activation(P16[:], pS[:], AF.Exp)
                                for qs in range(4):
                                    nc.tensor.matmul(
                                        em2p[qs][:],
                                        P16[:, qs * 128:(qs + 1) * 128],
                                        emoV[:, kt, :],
                                        start=(kt == 0),
                                        stop=(kt == N // 128 - 1))
                        for qs in range(4):
                            qt = qh * 4 + qs
                            rcp = pah.tile([128, 1], f32, tag="rcp")
                            nc.vector.reciprocal(rcp[:], em2p[qs][:, 300:301])
                            nc.vector.tensor_scalar(em2all[:, qt, :],
                                                    em2p[qs][:, 0:300],
                                                    rcp[:], None, op0=ALU.mult)
                # ---- head (per 128-query tile) ----
                with tc.tile_pool(name="ph2", bufs=2) as ph2, \
                     tc.tile_pool(name="psh", bufs=2, space="PSUM") as psh:
                    for qt in range(NQT):
                        em2 = em2all[:, qt, :]
                        ph_ = psh.tile([100, 128], f32, tag="phid")
                        for gc in range(3):
                            pe2 = psh.tile([100, 128], f16, tag="pe2t")
                            nc.tensor.matmul(pe2[:],
                                             em2[:, gc * 100:(gc + 1) * 100],
                                             id16[:], is_transpose=True,
                                             start=True, stop=True)
                            e2t = ph2.tile([100, 128], f16, tag="e2t")
                            nc.vector.tensor_copy(e2t[:], pe2[:])
                            nc.tensor.matmul(ph_[:], linw[:, gc, :], e2t[:],
                                             start=(gc == 0), stop=False)
                        nc.tensor.matmul(ph_[:], linb[:], ones[:, 0:128],
                                         start=False, stop=True)
                        hidT = ph2.tile([100, 128], f16, tag="hidT")
                        nc.scalar.activation(hidT[:], ph_[:], AF.Relu)
                        plg = psh.tile([NC, 128], f32, tag="plg")
                        nc.tensor.matmul(plg[:], smaxw[:], hidT[:], start=True,
                                         stop=False)
                        nc.tensor.matmul(plg[:], smaxb[:], ones[:, 0:128],
                                         start=False, stop=True)
                        lgT = ph2.tile([NC, 128], f32, tag="lgT")
                        nc.vector.tensor_copy(lgT[:], plg[:])
                        plt = psh.tile([128, NC], f32, tag="plt")
                        nc.tensor.matmul(plt[:], lgT[:], id32[0:NC, 0:NC],
                                         is_transpose=True, start=True,
                                         stop=True)
                        lg = ph2.tile([128, NC], f32, tag="lg")
                        nc.vector.tensor_copy(lg[:], plt[:])
                        m6 = ph2.tile([128, 1], f32, tag="m6")
                        nc.vector.reduce_max(m6[:], lg[:], axis=AX.XYZW)
                        nm6 = ph2.tile([128, 1], f32, tag="nm6")
                        nc.vector.tensor_scalar_mul(nm6[:], m6[:], -1.0)
                        e6 = ph2.tile([128, NC], f32, tag="e6")
                        s6 = ph2.tile([128, 1], f32, tag="s6")
                        nc.scalar.activation(e6[:], lg[:], AF.Exp, bias=nm6[:],
                                             scale=1.0, accum_out=s6[:])
                        ls6 = ph2.tile([128, 1], f32, tag="ls6")
                        nc.scalar.activation(ls6[:], s6[:], AF.Ln)
                        sh = ph2.tile([128, 1], f32, tag="sh")
                        nc.vector.tensor_add(sh[:], m6[:], ls6[:])
                        outt = ph2.tile([128, NC], f32, tag="outt")
                        nc.vector.tensor_scalar(outt[:], lg[:], sh[:], None,
                                                op0=ALU.subtract)
                        nc.sync.dma_start(outd[qt * 128:(qt + 1) * 128, :],
                                          outt[:])

    nc.compile()
    return nc


# ------------------------------------------------------------------ entry
def kernel(x, edge_index, edge_norm, edge_type, basis, comp, root_w, root_b,
           gc_rel_w, gc_rel_b, gc_root_w, beta_w, beta_b, lin_w, lin_b,
           smax_w, smax_b):
    x = np.ascontiguousarray(np.asarray(x, np.float32))
    per_core = _prep(edge_index, edge_type)

    if "k" not in _ker_cache:
        _ker_cache["k"] = _build()
    nc = _ker_cache["k"]

    xT = np.ascontiguousarray(x.T)
    shared = dict(
        xT=xT,
        xT2=np.ascontiguousarray(
            np.stack([xT[0:100], xT[100:200]], axis=1)),
        x16=np.asarray(x, np.float16),
        onesN=np.ones((1, N), np.float32),
        basis=np.ascontiguousarray(
            np.asarray(basis, np.float32).reshape(NB, F * H)),
        compT=np.ascontiguousarray(np.asarray(comp, np.float32).T),
        root_w=np.asarray(root_w, np.float32),
        root_b=np.asarray(root_b, np.float32).reshape(1, H),
        gc_rel_w=np.asarray(gc_rel_w, np.float32),
        gc_rel_b=np.asarray(gc_rel_b, np.float32).reshape(1, H),
        gc_root_w=np.asarray(gc_root_w, np.float32),
        beta_w=np.asarray(beta_w, np.float32),
        beta_b=np.asarray(beta_b, np.float32).reshape(1, 3 * H),
        lin_w16=np.asarray(lin_w, np.float16),
        lin_b=np.asarray(lin_b, np.float32).reshape(1, H),
        smax_w16=np.asarray(smax_w, np.float16),
        smax_b=np.asarray(smax_b, np.float32).reshape(1, NC),
        ident32=np.eye(128, dtype=np.float32),
        ident16=np.eye(128, dtype=np.float16),
        ones=np.ones((1, 512), np.float32),
    )
    in_maps = []
    for c in range(CORES):
        m = dict(shared)
        m["xTs"] = np.ascontiguousarray(x[c * NPC:(c + 1) * NPC, :].T)
        m.update(per_core[c])
        in_maps.append(m)

    res = run_bass_kernel_spmd(nc, in_maps, core_ids=list(range(CORES)),
                               trace_cores=[0])
    global _last_res
    _last_res = res
    return np.concatenate([res.results[c]["out"] for c in range(CORES)], axis=0)
